# revision 20
# baseline (speedup 1.0000x reference)
"""Trainium2 Bass kernel for nn_ChannelWiseConv (depthwise conv stack + KAN head).

Strategy (per core, pure data parallelism over batch):
  - 256 images/core, fp16 conv path (PE runs 1 cycle/row vs fp32's 4; HBM
    traffic halves). Each stride-2 depthwise 3x3 conv is 3 PE matmuls
    accumulating in fp32 PSUM: contraction over input rows with per-channel
    banded weight matrices (one per kernel-column tap); column subsampling is
    in the moving-operand access pattern. Channels are blocked (2/4/8/13 per
    matmul as planes shrink) to keep K near 128.
  - groups are processed in PAIRS for conv2..conv6 (doubling matmul N and
    halving LDWEIGHTS/instruction counts), and two channel-blocks share one
    128-partition PSUM bank via the PE output partition offset, so each
    bias+relu drains a full [128, N] tile in one op. Relu ops alternate
    between the Scalar(ACT) and Vector(DVE) engines.
  - KAN: the knot-offset matrix D[img, j] = u[ch_j, img] - k_j comes from one
    fp16 PE matmul against an augmented [u; 1] stationary; Cox-de-Boor runs
    on the Vector engine in fp16 (2x DVE rate); silu is computed as
    x/(1+exp(-x)) so every ACT function lives in one activation-table set
    (no 1.3us table reloads in the critical chain). KAN tile 0 is staged
    across conv pairs 4-7; only tile 1's chain is a serial tail.
    log_softmax on-chip.
"""

import numpy as np

IN_CH, HIDDEN, NCLS = 13, 20, 10
B_FULL, NCORE = 2048, 8
B = B_FULL // NCORE          # images per core
NG = 16                      # image groups per core
GI = B // NG                 # images per group (16)
NP = NG // 2                 # group pairs
H_GRID = 0.4                 # KAN knot spacing; u = (x + 2.2) / 0.4

CG1 = [(0, 2), (2, 2), (4, 2), (6, 2), (8, 2), (10, 2), (12, 1)]
CG2 = [(0, 4), (4, 4), (8, 4), (12, 1)]
CG3 = [(0, 8), (8, 5)]

# fp16 const pack column offsets
O_B1, O_B2, O_B3 = 0, 1344, 2112
O_B4, O_B5, O_B6 = 2496, 2652, 2730
O_C1A, O_C1B, O_C2S = 2756, 2776, 2796
O_C2B1, O_C2B2 = 2806, 2816
O_PD, O_PD2 = 2826, 2982
W16 = 3222
# fp32 const pack column offsets
O_BR1, O_BR2, O_BR3 = 0, 4, 6
O_BV4, O_BV5, O_UB = 7, 8, 9
O_IDEN = 10
O_MISC = 138
W32 = 141

_BUILT = None  # cached (nc, input_names)


# ----------------------------------------------------------------------------
# host-side constant packing
# ----------------------------------------------------------------------------

def _pack_bands(w, S, cgs, slotM, rows):
    So = S // 2
    out = np.zeros((rows, len(cgs) * 3 * slotM), np.float32)
    for gi, (c0, nch) in enumerate(cgs):
        for b in range(3):
            col0 = (gi * 3 + b) * slotM
            for cl in range(nch):
                for i in range(So):
                    for a in range(3):
                        r = 2 * i + a - 1
                        if 0 <= r < S:
                            out[cl * S + r, col0 + cl * So + i] = w[c0 + cl, 0, a, b]
    return out


def _pack_bias(bias, cgs, So, rows):
    out = np.zeros((rows, len(cgs)), np.float32)
    for gi, (c0, nch) in enumerate(cgs):
        for cl in range(nch):
            out[cl * So:(cl + 1) * So, gi] = bias[c0 + cl]
    return out


def _host_consts(inp):
    p16 = np.zeros((128, W16), np.float32)
    p16[:, O_B1:O_B1 + 1344] = _pack_bands(inp["w1"], 64, CG1, 64, 128)
    p16[:, O_B2:O_B2 + 768] = _pack_bands(inp["w2"], 32, CG2, 64, 128)
    p16[:, O_B3:O_B3 + 384] = _pack_bands(inp["w3"], 16, CG3, 64, 128)
    p16[0:104, O_B4:O_B4 + 156] = _pack_bands(inp["w4"], 8, [(0, 13)], 52, 104)
    p16[0:52, O_B5:O_B5 + 78] = _pack_bands(inp["w5"], 4, [(0, 13)], 26, 52)
    b6m = np.zeros((26, 26), np.float32)
    for bb in range(2):
        for ch in range(13):
            for r in range(2):
                b6m[ch * 2 + r, bb * 13 + ch] = inp["w6"][ch, 0, r, bb]
    p16[0:26, O_B6:O_B6 + 26] = b6m
    # KAN layer 1 rhs pieces: silu part [13,20]; basis+bias part [105,20]
    p16[0:13, O_C1A:O_C1A + 20] = inp["sb1"]
    c1b = np.zeros((105, HIDDEN), np.float32)
    for n in range(8):
        for i in range(13):
            c1b[n * 13 + i] = inp["coef1"][i, :, n] * inp["ss1"][i] / 6.0
    c1b[104] = inp["bias1"]
    p16[0:105, O_C1B:O_C1B + 20] = c1b
    # KAN layer 2: silu [20,10]; basis rows 0..107 [108,10]; 108..159+bias [53,10]
    p16[0:20, O_C2S:O_C2S + 10] = inp["sb2"]
    c2b = np.zeros((161, NCLS), np.float32)
    for n in range(8):
        for i in range(20):
            c2b[n * 20 + i] = inp["coef2"][i, :, n] * inp["ss2"][i] / 6.0
    c2b[160] = inp["bias2"]
    p16[0:108, O_C2B1:O_C2B1 + 10] = c2b[0:108]
    p16[0:53, O_C2B2:O_C2B2 + 10] = c2b[108:161]
    # D = [u;1].T @ P_D gives D[img, k*13+ch] = u[ch,img] - k in one matmul
    pd = np.zeros((14, 156), np.float32)
    for k in range(12):
        for ch in range(13):
            pd[ch, k * 13 + ch] = 1.0
            pd[13, k * 13 + ch] = float(-k)
    p16[0:14, O_PD:O_PD + 156] = pd
    # D2 = [h1;1].T @ P_D2 with the u2 = 2.5*h1 + 5.5 affine folded in
    pd2 = np.zeros((21, 240), np.float32)
    for k in range(12):
        for ch in range(20):
            pd2[ch, k * 20 + ch] = 2.5
            pd2[20, k * 20 + ch] = 5.5 - float(k)
    p16[0:21, O_PD2:O_PD2 + 240] = pd2

    p32 = np.zeros((128, W32), np.float32)
    bv1 = _pack_bias(inp["b1"], CG1, 32, 64)
    for j in range(3):
        p32[0:64, O_BR1 + j] = bv1[:, 2 * j]
        p32[64:128, O_BR1 + j] = bv1[:, 2 * j + 1]
    p32[0:32, O_BR1 + 3] = bv1[0:32, 6]
    bv2 = _pack_bias(inp["b2"], CG2, 16, 64)
    p32[0:64, O_BR2] = bv2[:, 0]
    p32[64:128, O_BR2] = bv2[:, 1]
    p32[0:64, O_BR2 + 1] = bv2[:, 2]
    p32[64:80, O_BR2 + 1] = bv2[0:16, 3]
    bv3 = _pack_bias(inp["b3"], CG3, 8, 64)
    p32[0:64, O_BR3] = bv3[:, 0]
    p32[64:104, O_BR3] = bv3[0:40, 1]
    p32[0:52, O_BV4:O_BV4 + 1] = _pack_bias(inp["b4"], [(0, 13)], 4, 52)
    p32[0:26, O_BV5:O_BV5 + 1] = _pack_bias(inp["b5"], [(0, 13)], 2, 26)
    p32[0:13, O_UB:O_UB + 1] = (2.5 * (inp["b6"] + 2.2)).reshape(13, 1)
    p32[:, O_IDEN:O_IDEN + 128] = np.eye(128, dtype=np.float32)
    p32[:, O_MISC] = 2.2
    p32[:, O_MISC + 1] = 0.0
    p32[:, O_MISC + 2] = -2.2
    return {"c16": p16.astype(np.float16), "c32": p32}


def _shard_x(x_shard):
    # [256,13,64,64] -> xa [NG,128,6,GI,64] fp16 (partition = pair-slot*64+row),
    # xb [NG,64,GI,64] fp16 (ch 12)
    xs = x_shard.reshape(NG, GI, 13, 64, 64)
    a = xs[:, :, 0:12].reshape(NG, GI, 6, 2, 64, 64)
    a = a.transpose(0, 3, 4, 2, 1, 5).reshape(NG, 128, 6, GI, 64)
    xa = np.ascontiguousarray(a, dtype=np.float16)
    xb = np.ascontiguousarray(xs[:, :, 12].transpose(0, 2, 1, 3), dtype=np.float16)
    return xa, xb


# ----------------------------------------------------------------------------
# bass program
# ----------------------------------------------------------------------------

def _build():
    global _BUILT
    if _BUILT is not None:
        return _BUILT
    from contextlib import ExitStack
    import concourse.bass as bass  # noqa: F401
    import concourse.bacc as bacc
    import concourse.tile as tile
    import concourse.mybir as mybir

    f32 = mybir.dt.float32
    f16 = mybir.dt.float16
    AF = mybir.ActivationFunctionType
    OP = mybir.AluOpType
    AX = mybir.AxisListType

    nc = bacc.Bacc("TRN2")
    T = nc.tensor

    d_xa = nc.dram_tensor("xa", [NG, 128, 6, GI, 64], f16, kind="ExternalInput")
    d_xb = nc.dram_tensor("xb", [NG, 64, GI, 64], f16, kind="ExternalInput")
    d_c16 = nc.dram_tensor("c16", [128, W16], f16, kind="ExternalInput")
    d_c32 = nc.dram_tensor("c32", [128, W32], f32, kind="ExternalInput")
    d_out = nc.dram_tensor("out", [B, NCLS], f32, kind="ExternalOutput")

    PGI = 2 * GI  # images per pair

    with tile.TileContext(nc) as tc, ExitStack() as ctx:
        cpool = ctx.enter_context(tc.tile_pool(name="consts", bufs=1))
        c16 = cpool.tile([128, W16], f16, name="t_c16")
        c32 = cpool.tile([128, W32], f32, name="t_c32")
        # t_u rows 0-12: per-channel u values; row 13: ones (for the D matmul)
        t_u = cpool.tile([14, B], f16, name="t_u")

        p_x1 = ctx.enter_context(tc.tile_pool(name="x1", bufs=4))
        p_x2 = ctx.enter_context(tc.tile_pool(name="x2", bufs=2))
        p_x3 = ctx.enter_context(tc.tile_pool(name="x3", bufs=2))
        p_sm = ctx.enter_context(tc.tile_pool(name="xsm", bufs=2))
        kpool = ctx.enter_context(tc.tile_pool(name="kan", bufs=1))

        pp1 = ctx.enter_context(tc.tile_pool(name="ps1", bufs=3, space="PSUM"))
        pp2 = ctx.enter_context(tc.tile_pool(name="ps2", bufs=2, space="PSUM"))
        pp3 = ctx.enter_context(tc.tile_pool(name="ps3", bufs=1, space="PSUM"))
        pp456 = ctx.enter_context(tc.tile_pool(name="ps456", bufs=1, space="PSUM"))
        ppk = ctx.enter_context(tc.tile_pool(name="psk", bufs=1, space="PSUM"))

        def scrap_mm(src):
            # scrap matmul absorbing a DMA wait on the PE queue; shares the
            # conv3 PSUM bank (write-only, no readers -> no real conflicts)
            s = pp3.tile([104, PGI * 8], f32, tag="ps3", name="scrap")
            T.matmul(s[0:1, 0:1], src, src, start=True, stop=True)

        def bias_relu(e, dest, src, bap):
            if e == "s":
                nc.scalar.activation(dest, src, AF.Relu, bias=bap)
            else:
                nc.vector.tensor_scalar(dest, src, bap, 0.0,
                                        op0=OP.add, op1=OP.max)

        def group_dma(g):
            xt = p_x1.tile([128, 6 * GI * 64], f16, tag="x1", name="xt")
            h = 3 * GI * 64
            nc.sync.dma_start(
                xt[:, 0:h].rearrange("p (c i w) -> p c i w", c=3, w=64),
                d_xa[g, :, 0:3])
            nc.sync.dma_start(
                xt[:, h:2 * h].rearrange("p (c i w) -> p c i w", c=3, w=64),
                d_xa[g, :, 3:6])
            xtb = p_x1.tile([64, GI * 64], f16, tag="xb", name="xtb")
            nc.sync.dma_start(
                xtb[:, :].rearrange("p (i w) -> p i w", w=64), d_xb[g])
            return xt, xtb

        def conv1_group(xt, xtb, x2t, gq):
            # one group's conv1; channel-group pairs share a [128,512] PSUM
            # tile via the PE output partition offset
            csl = slice(gq * GI * 32, (gq + 1) * GI * 32)
            for j in range(4):
                cgs = [2 * j] if j == 3 else [2 * j, 2 * j + 1]
                ps = pp1.tile([128, GI * 32], f32, tag="ps1", name="ps1t")
                Mtot = 0
                for q, cg in enumerate(cgs):
                    nch = CG1[cg][1]
                    K, M = nch * 64, nch * 32
                    Mtot = 64 * q + M
                    if nch == 2:
                        xv = xt[0:K, cg * GI * 64:(cg + 1) * GI * 64].rearrange(
                            "p (i w) -> p i w", w=64)
                    else:
                        xv = xtb[0:K, :].rearrange("p (i w) -> p i w", w=64)
                    pv = ps[64 * q:64 * q + M, :].rearrange(
                        "p (i w) -> p i w", w=32)
                    lo = lambda b: O_B1 + (cg * 3 + b) * 64
                    T.matmul(pv, c16[0:K, lo(1):lo(1) + M],
                             xv[:, :, 0:64:2], start=True, stop=False)
                    T.matmul(pv, c16[0:K, lo(2):lo(2) + M],
                             xv[:, :, 1:64:2], start=False, stop=False)
                    T.matmul(pv[:, :, 1:32], c16[0:K, lo(0):lo(0) + M],
                             xv[:, :, 1:62:2], start=False, stop=True,
                             skip_group_check=True)
                bias_relu("s" if (j + gq) % 2 else "v",
                          x2t[j][0:Mtot, csl], ps[0:Mtot, :],
                          c32[0:Mtot, O_BR1 + j:O_BR1 + j + 1])

        # group-0/1 input DMAs + consts first; c16 gates the first matmul
        nc.sync.dma_start(c16[:, :], d_c16[:, :])
        g0_t = group_dma(0)
        nc.sync.dma_start(c32[:, :], d_c32[:, :])
        g01_tiles = [g0_t, group_dma(1)]
        scrap_mm(c16[0:1, 0:1])
        scrap_mm(c32[0:1, 0:1])
        # ones row for the KAN D matmul; conv6 overwrites rows 0-12
        nc.vector.memset(t_u[0:14, :], 1.0)

        # ------------------------------ KAN head ------------------------------
        def emit_kan(t):
            """Generator: one KAN tile (128 images) in stages; yields between
            stages so tile 0 can interleave with conv pairs 4-7."""
            sl = slice(t * 128, (t + 1) * 128)
            ps_D = ppk.tile([128, 256], f32, tag="kps", name="ps_D")
            T.matmul(ps_D[0:128, 0:156], t_u[0:14, sl],
                     c16[0:14, O_PD:O_PD + 156], start=True, stop=True)
            D = kpool.tile([128, 156], f16, tag="D", name="Dt")
            nc.vector.tensor_copy(D[:, :], ps_D[0:128, 0:156])
            ge = kpool.tile([128, 143], f16, tag="ge", name="ge")
            lt = kpool.tile([128, 143], f16, tag="lt", name="lt")
            nc.vector.tensor_scalar(ge[:, :], D[:, 0:143], 0.0, None, op0=OP.is_ge)
            nc.vector.tensor_scalar(lt[:, :], D[:, 13:156], 0.0, None, op0=OP.is_lt)
            Bc = kpool.tile([128, 143], f16, tag="B0", name="Bc")
            nc.vector.tensor_mul(Bc[:, :], ge[:, :], lt[:, :])
            yield
            # Cox-de-Boor levels (unnormalized; /6 folded into c1b); the last
            # level is written fp32 because it feeds an fp32 PE transpose
            wid = 143
            for p in range(1, 4):
                wid -= 13
                dt_l = f16 if p < 3 else f32
                ta = kpool.tile([128, wid], dt_l, tag=f"ta{p}", name="ta")
                tb = kpool.tile([128, wid], dt_l, tag=f"tb{p}", name="tb")
                nc.vector.tensor_mul(ta[:, :], D[:, 0:wid], Bc[:, 0:wid])
                nc.vector.tensor_mul(tb[:, :], D[:, 13 * (p + 1):13 * (p + 1) + wid],
                                     Bc[:, 13:13 + wid])
                if p < 3:
                    Bc = kpool.tile([128, wid], f16, tag=f"B{p}", name="Bc")
                    nc.vector.tensor_sub(Bc[:, :], ta[:, :], tb[:, :])
                else:
                    Bc = kpool.tile([128, 105], f32, tag="B3", name="Bc")
                    nc.vector.tensor_sub(Bc[:, 0:104], ta[:, :], tb[:, :])
                    nc.vector.memset(Bc[:, 104:105], 1.0)
            # stkA = silu(x) = x / (1 + exp(-x)), x = 0.4*u - 2.2; keeps every
            # ACT function in the natural_log_exp table set (no reloads)
            xA = kpool.tile([13, 128], f32, tag="xA", name="xA")
            nc.vector.tensor_scalar(xA[:, :], t_u[0:13, sl], H_GRID, -2.2,
                                    op0=OP.mult, op1=OP.add)
            eA = kpool.tile([13, 128], f32, tag="eA", name="eA")
            nc.scalar.activation(eA[:, :], t_u[0:13, sl], AF.Exp,
                                 bias=c32[0:13, O_MISC:O_MISC + 1],
                                 scale=-H_GRID)
            dA = kpool.tile([13, 128], f32, tag="dA", name="dA")
            nc.vector.tensor_scalar(dA[:, :], eA[:, :], 1.0, None, op0=OP.add)
            rA = kpool.tile([13, 128], f32, tag="rA", name="rA")
            nc.vector.reciprocal(rA[:, :], dA[:, :])
            stkA = kpool.tile([13, 128], f16, tag="stkA", name="stkA")
            nc.vector.tensor_mul(stkA[:, :], xA[:, :], rA[:, :])
            yield
            stkB = kpool.tile([105, 128], f16, tag="stkB", name="stkB")
            ps_b1 = ppk.tile([128, 256], f32, tag="kps", name="ps_b1")
            T.transpose(ps_b1[0:105, 0:128], Bc[:, 0:105],
                        c32[:, O_IDEN:O_IDEN + 128])
            nc.scalar.copy(stkB[:, :], ps_b1[0:105, 0:128])
            yield
            ps_h1 = ppk.tile([128, 256], f32, tag="kps", name="ps_h1")
            T.matmul(ps_h1[0:128, 0:20], stkA[:, :], c16[0:13, O_C1A:O_C1A + 20],
                     start=True, stop=False)
            T.matmul(ps_h1[0:128, 0:20], stkB[:, :], c16[0:105, O_C1B:O_C1B + 20],
                     start=False, stop=True)
            yield
            # ---- KAN layer 2 (u2 = 2.5*h1 + 5.5 folded into P_D2) ----
            h1 = kpool.tile([128, 20], f32, tag="h1", name="h1")
            nc.scalar.copy(h1[:, :], ps_h1[0:128, 0:20])
            ps_t2 = ppk.tile([128, 256], f32, tag="kps", name="ps_t2")
            T.transpose(ps_t2[0:20, 0:128], h1[:, :], c32[:, O_IDEN:O_IDEN + 128])
            stk2u = kpool.tile([21, 128], f16, tag="s2u", name="stk2u")
            nc.vector.memset(stk2u[0:21, :], 1.0)
            nc.scalar.copy(stk2u[0:20, :], ps_t2[0:20, 0:128])
            # s2s = silu(h1) (the reference's base term for layer 2)
            e2 = kpool.tile([20, 128], f32, tag="e2", name="e2")
            nc.scalar.activation(e2[:, :], stk2u[0:20, :], AF.Exp,
                                 bias=c32[0:20, O_MISC + 1:O_MISC + 2],
                                 scale=-1.0)
            d2 = kpool.tile([20, 128], f32, tag="d2", name="d2")
            nc.vector.tensor_scalar(d2[:, :], e2[:, :], 1.0, None, op0=OP.add)
            r2 = kpool.tile([20, 128], f32, tag="r2", name="r2")
            nc.vector.reciprocal(r2[:, :], d2[:, :])
            stk2s = kpool.tile([20, 128], f16, tag="s2s", name="stk2s")
            nc.vector.tensor_mul(stk2s[:, :], stk2u[0:20, :], r2[:, :])
            ps_D2 = ppk.tile([128, 256], f32, tag="kps", name="ps_D2")
            T.matmul(ps_D2[0:128, 0:240], stk2u[0:21, :],
                     c16[0:21, O_PD2:O_PD2 + 240], start=True, stop=True)
            D2 = kpool.tile([128, 240], f16, tag="D2", name="D2t")
            nc.vector.tensor_copy(D2[:, :], ps_D2[0:128, 0:240])
            ge2 = kpool.tile([128, 220], f16, tag="ge2", name="ge2")
            lt2 = kpool.tile([128, 220], f16, tag="lt2", name="lt2")
            nc.vector.tensor_scalar(ge2[:, :], D2[:, 0:220], 0.0, None, op0=OP.is_ge)
            nc.vector.tensor_scalar(lt2[:, :], D2[:, 20:240], 0.0, None, op0=OP.is_lt)
            Bc2 = kpool.tile([128, 220], f16, tag="B0_2", name="Bc2")
            nc.vector.tensor_mul(Bc2[:, :], ge2[:, :], lt2[:, :])
            yield
            wid = 220
            for p in range(1, 4):
                wid -= 20
                dt_l = f16 if p < 3 else f32
                ta = kpool.tile([128, wid], dt_l, tag=f"t2a{p}", name="ta2")
                tb = kpool.tile([128, wid], dt_l, tag=f"t2b{p}", name="tb2")
                nc.vector.tensor_mul(ta[:, :], D2[:, 0:wid], Bc2[:, 0:wid])
                nc.vector.tensor_mul(tb[:, :], D2[:, 20 * (p + 1):20 * (p + 1) + wid],
                                     Bc2[:, 20:20 + wid])
                if p < 3:
                    Bc2 = kpool.tile([128, wid], f16, tag=f"B{p}_2", name="Bc2")
                    nc.vector.tensor_sub(Bc2[:, :], ta[:, :], tb[:, :])
                else:
                    Bc2 = kpool.tile([128, 161], f32, tag="B3_2", name="Bc2")
                    nc.vector.tensor_sub(Bc2[:, 0:160], ta[:, :], tb[:, :])
                    nc.vector.memset(Bc2[:, 160:161], 1.0)
            yield
            stk2a = kpool.tile([108, 128], f16, tag="s2a", name="stk2a")
            stk2b = kpool.tile([53, 128], f16, tag="s2b", name="stk2b")
            ps_b2 = ppk.tile([128, 256], f32, tag="kps", name="ps_b2")
            T.transpose(ps_b2[0:108, 0:128], Bc2[:, 0:108],
                        c32[:, O_IDEN:O_IDEN + 128])
            nc.scalar.copy(stk2a[:, :], ps_b2[0:108, 0:128])
            ps_b3 = ppk.tile([128, 256], f32, tag="kps", name="ps_b3")
            T.transpose(ps_b3[0:53, 0:128], Bc2[:, 108:161],
                        c32[:, O_IDEN:O_IDEN + 128])
            nc.vector.tensor_copy(stk2b[:, :], ps_b3[0:53, 0:128])
            yield
            ps_lg = ppk.tile([128, 256], f32, tag="kps", name="ps_lg")
            T.matmul(ps_lg[0:128, 0:NCLS], stk2a[:, :],
                     c16[0:108, O_C2B1:O_C2B1 + 10], start=True, stop=False)
            T.matmul(ps_lg[0:128, 0:NCLS], stk2s[:, :],
                     c16[0:20, O_C2S:O_C2S + 10], start=False, stop=False)
            T.matmul(ps_lg[0:128, 0:NCLS], stk2b[:, :],
                     c16[0:53, O_C2B2:O_C2B2 + 10], start=False, stop=True)
            yield
            # ---- log_softmax (on an SBUF copy) ----
            lg_s = kpool.tile([128, NCLS], f32, tag="lg_s", name="lg_s")
            nc.vector.tensor_copy(lg_s[:, :], ps_lg[0:128, 0:NCLS])
            negm = kpool.tile([128, 1], f32, tag="negm", name="negm")
            nc.vector.reduce_max(negm[:, :], lg_s[:, :], axis=AX.X, negate=True)
            ex = kpool.tile([128, NCLS], f32, tag="ex", name="ex")
            nc.scalar.activation(ex[:, :], lg_s[:, :], AF.Exp, bias=negm[:, 0:1])
            ssum = kpool.tile([128, 1], f32, tag="ssum", name="ssum")
            nc.vector.reduce_sum(ssum[:, :], ex[:, :], axis=AX.X)
            lsum = kpool.tile([128, 1], f32, tag="lsum", name="lsum")
            nc.scalar.activation(lsum[:, :], ssum[:, :], AF.Ln,
                                 bias=c32[0:128, O_MISC + 1:O_MISC + 2])
            res = kpool.tile([128, NCLS], f32, tag="res", name="res")
            nc.vector.tensor_scalar(res[:, :], lg_s[:, :], negm[:, 0:1],
                                    lsum[:, 0:1], op0=OP.add, op1=OP.subtract)
            nc.sync.dma_start(d_out[sl, :], res[:, :])

        kan_gens = []
        for gp in range(NP):
            g0, g1 = 2 * gp, 2 * gp + 1
            pr = [g01_tiles[0], g01_tiles[1]] if gp == 0 else \
                 [group_dma(g0), group_dma(g1)]
            psl = slice(g0 * GI, (g1 + 1) * GI)  # pair image slice

            # ---- conv1 (per group) -> x2 pair tiles ----
            x2t = [p_x2.tile([128, PGI * 32], f16, tag=f"x2_{k}", name=f"x2_{k}")
                   for k in range(4)]
            for gq in range(2):
                xt, xtb = pr[gq]
                h = 3 * GI * 64
                scrap_mm(xt[0:1, 0:1])
                scrap_mm(xt[0:1, h:h + 1])
                scrap_mm(xtb[0:1, 0:1])
                conv1_group(xt, xtb, x2t, gq)

            # ---- conv2 on the pair -> x3 tiles ----
            x3t = [p_x3.tile([128, PGI * 16], f16, tag=f"x3_{k}", name=f"x3_{k}")
                   for k in range(2)]
            for j in range(2):
                ps = pp2.tile([128, PGI * 16], f32, tag="ps2", name="ps2t")
                Mtot = 0
                for q, k4 in enumerate((2 * j, 2 * j + 1)):
                    nch = CG2[k4][1]
                    K, M = nch * 32, nch * 16
                    Mtot = 64 * q + M
                    xv = x2t[k4][0:K, :].rearrange("p (i w) -> p i w", w=32)
                    pv = ps[64 * q:64 * q + M, :].rearrange(
                        "p (i w) -> p i w", w=16)
                    lo = lambda b: O_B2 + (k4 * 3 + b) * 64
                    T.matmul(pv, c16[0:K, lo(1):lo(1) + M],
                             xv[:, :, 0:32:2], start=True, stop=False)
                    T.matmul(pv, c16[0:K, lo(2):lo(2) + M],
                             xv[:, :, 1:32:2], start=False, stop=False)
                    T.matmul(pv[:, :, 1:16], c16[0:K, lo(0):lo(0) + M],
                             xv[:, :, 1:30:2], start=False, stop=True,
                             skip_group_check=True)
                bias_relu("s" if (j + gp) % 2 else "v",
                          x3t[j][0:Mtot, :], ps[0:Mtot, :],
                          c32[0:Mtot, O_BR2 + j:O_BR2 + j + 1])

            # ---- conv3 on the pair -> x4 [104, PGI*8] ----
            x4 = p_sm.tile([104, PGI * 8], f16, tag="x4", name="x4")
            ps3 = pp3.tile([104, PGI * 8], f32, tag="ps3", name="ps3t")
            for q, k8 in enumerate((0, 1)):
                nch = CG3[k8][1]
                K, M = nch * 16, nch * 8
                xv = x3t[k8][0:K, :].rearrange("p (i w) -> p i w", w=16)
                pv = ps3[64 * q:64 * q + M, :].rearrange("p (i w) -> p i w", w=8)
                lo = lambda b: O_B3 + (k8 * 3 + b) * 64
                T.matmul(pv, c16[0:K, lo(1):lo(1) + M],
                         xv[:, :, 0:16:2], start=True, stop=False)
                T.matmul(pv, c16[0:K, lo(2):lo(2) + M],
                         xv[:, :, 1:16:2], start=False, stop=False)
                T.matmul(pv[:, :, 1:8], c16[0:K, lo(0):lo(0) + M],
                         xv[:, :, 1:14:2], start=False, stop=True,
                         skip_group_check=True)
            bias_relu("s" if gp % 2 else "v", x4[0:104, :], ps3[0:104, :],
                      c32[0:104, O_BR3:O_BR3 + 1])

            # ---- conv4 [104 -> 52] ----
            x5 = p_sm.tile([52, PGI * 4], f16, tag="x5", name="x5")
            xv = x4[0:104, :].rearrange("p (i w) -> p i w", w=8)
            ps4 = pp456.tile([64, PGI * 4], f32, tag="ps456", name="ps4t")
            pv = ps4[0:52, :].rearrange("p (i w) -> p i w", w=4)
            T.matmul(pv, c16[0:104, O_B4 + 52:O_B4 + 104], xv[:, :, 0:8:2],
                     start=True, stop=False)
            T.matmul(pv, c16[0:104, O_B4 + 104:O_B4 + 156], xv[:, :, 1:8:2],
                     start=False, stop=False)
            T.matmul(pv[:, :, 1:4], c16[0:104, O_B4:O_B4 + 52],
                     xv[:, :, 1:6:2], start=False, stop=True,
                     skip_group_check=True)
            bias_relu("v", x5[:, :], ps4[0:52, :], c32[0:52, O_BV4:O_BV4 + 1])

            # ---- conv5 [52 -> 26] ----
            x6 = p_sm.tile([26, PGI * 2], f16, tag="x6", name="x6")
            xv = x5[0:52, :].rearrange("p (i w) -> p i w", w=4)
            ps5 = pp456.tile([64, PGI * 2], f32, tag="ps456", name="ps5t")
            pv = ps5[0:26, :].rearrange("p (i w) -> p i w", w=2)
            T.matmul(pv, c16[0:52, O_B5 + 26:O_B5 + 52], xv[:, :, 0:4:2],
                     start=True, stop=False)
            T.matmul(pv, c16[0:52, O_B5 + 52:O_B5 + 78], xv[:, :, 1:4:2],
                     start=False, stop=False)
            T.matmul(pv[:, :, 1:2], c16[0:52, O_B5:O_B5 + 26],
                     xv[:, :, 1:2:2], start=False, stop=True,
                     skip_group_check=True)
            bias_relu("s", x6[:, :], ps5[0:26, :], c32[0:26, O_BV5:O_BV5 + 1])

            # ---- conv6 (2x2 valid) -> u[:, pair slice] ----
            xv = x6[0:26, :].rearrange("p (i w) -> p i w", w=2)
            ps6 = pp456.tile([64, PGI], f32, tag="ps456", name="ps6t")
            T.matmul(ps6[0:13, :], c16[0:26, O_B6:O_B6 + 13],
                     xv[:, :, 0:1], start=True, stop=False)
            T.matmul(ps6[0:13, :], c16[0:26, O_B6 + 13:O_B6 + 26],
                     xv[:, :, 1:2], start=False, stop=True)
            nc.vector.tensor_scalar(t_u[0:13, psl], ps6[0:13, :],
                                    2.5, c32[0:13, O_UB:O_UB + 1],
                                    op0=OP.mult, op1=OP.add)

            if gp == 3:
                kan_gens.append(emit_kan(0))
            for gen in kan_gens:
                next(gen, None)
                next(gen, None)

        kan_gens.append(emit_kan(1))
        for gen in kan_gens:
            for _ in gen:
                pass

    nc.compile()  # bacc lowering: wait splitting via event semaphores, etc.
    _BUILT = (nc, ["xa", "xb", "c16", "c32"])
    return _BUILT


# ----------------------------------------------------------------------------
# entry point
# ----------------------------------------------------------------------------

def kernel(**inputs):
    from concourse import bass_utils

    x = np.asarray(inputs["x"], np.float32)
    cons = _host_consts({k: np.asarray(v, np.float32)
                         for k, v in inputs.items() if k != "x"})
    nc, _names = _build()

    in_maps = []
    for core in range(NCORE):
        xa, xb = _shard_x(x[core * B:(core + 1) * B])
        in_maps.append({"xa": xa, "xb": xb, **cons})
    res = bass_utils.run_bass_kernel_spmd(nc, in_maps, core_ids=list(range(NCORE)))
    global LAST_RES
    LAST_RES = res
    return np.concatenate([r["out"] for r in res.results], axis=0)


LAST_RES = None


# revision 21
# speedup vs baseline: 1.1566x; 1.1566x over previous
"""Trainium2 Bass kernel for nn_ChannelWiseConv (depthwise conv stack + KAN head).

Strategy (per core, pure data parallelism over batch):
  - 256 images/core, fp16 conv path (PE runs 1 cycle/row vs fp32's 4; HBM
    traffic halves). Each stride-2 depthwise 3x3 conv is 3 PE matmuls
    accumulating in fp32 PSUM: contraction over input rows with per-channel
    banded weight matrices (one per kernel-column tap); column subsampling is
    in the moving-operand access pattern. Channels are blocked (2/4/8/13 per
    matmul as planes shrink) to keep K near 128.
  - groups are processed in PAIRS for conv2..conv6 (doubling matmul N and
    halving LDWEIGHTS/instruction counts), and two channel-blocks share one
    128-partition PSUM bank via the PE output partition offset, so each
    bias+relu drains a full [128, N] tile in one op. Relu ops alternate
    between the Scalar(ACT) and Vector(DVE) engines.
  - KAN: the knot-offset matrix D[img, j] = u[ch_j, img] - k_j comes from one
    fp16 PE matmul against an augmented [u; 1] stationary; Cox-de-Boor runs
    on the Vector engine in fp16 (2x DVE rate); silu is computed as
    x/(1+exp(-x)) so every ACT function lives in one activation-table set
    (no 1.3us table reloads in the critical chain). KAN tile 0 is staged
    across conv pairs 4-7; only tile 1's chain is a serial tail.
    log_softmax on-chip.
"""

import numpy as np

IN_CH, HIDDEN, NCLS = 13, 20, 10
B_FULL, NCORE = 2048, 8
B = B_FULL // NCORE          # images per core
NG = 16                      # image groups per core
GI = B // NG                 # images per group (16)
NP = NG // 2                 # group pairs
H_GRID = 0.4                 # KAN knot spacing; u = (x + 2.2) / 0.4

CG1 = [(0, 2), (2, 2), (4, 2), (6, 2), (8, 2), (10, 2), (12, 1)]
CG2 = [(0, 4), (4, 4), (8, 4), (12, 1)]
CG3 = [(0, 8), (8, 5)]

# fp16 const pack column offsets
O_B1, O_B2, O_B3 = 0, 1344, 2112
O_B4, O_B5, O_B6 = 2496, 2652, 2730
O_C1A, O_C1B, O_C2S = 2756, 2776, 2796
O_C2B1, O_C2B2 = 2806, 2816
O_PD, O_PD2 = 2826, 2982
W16 = 3222
# fp32 const pack column offsets
O_BR1, O_BR2, O_BR3 = 0, 4, 6
O_BV4, O_BV5, O_UB = 7, 8, 9
O_IDEN = 10
O_MISC = 138
W32 = 141

_BUILT = None  # cached (nc, input_names)


# ----------------------------------------------------------------------------
# host-side constant packing
# ----------------------------------------------------------------------------

def _pack_bands(w, S, cgs, slotM, rows):
    So = S // 2
    out = np.zeros((rows, len(cgs) * 3 * slotM), np.float32)
    for gi, (c0, nch) in enumerate(cgs):
        for b in range(3):
            col0 = (gi * 3 + b) * slotM
            for cl in range(nch):
                for i in range(So):
                    for a in range(3):
                        r = 2 * i + a - 1
                        if 0 <= r < S:
                            out[cl * S + r, col0 + cl * So + i] = w[c0 + cl, 0, a, b]
    return out


def _pack_bias(bias, cgs, So, rows):
    out = np.zeros((rows, len(cgs)), np.float32)
    for gi, (c0, nch) in enumerate(cgs):
        for cl in range(nch):
            out[cl * So:(cl + 1) * So, gi] = bias[c0 + cl]
    return out


def _host_consts(inp):
    p16 = np.zeros((128, W16), np.float32)
    p16[:, O_B1:O_B1 + 1344] = _pack_bands(inp["w1"], 64, CG1, 64, 128)
    p16[:, O_B2:O_B2 + 768] = _pack_bands(inp["w2"], 32, CG2, 64, 128)
    p16[:, O_B3:O_B3 + 384] = _pack_bands(inp["w3"], 16, CG3, 64, 128)
    p16[0:104, O_B4:O_B4 + 156] = _pack_bands(inp["w4"], 8, [(0, 13)], 52, 104)
    p16[0:52, O_B5:O_B5 + 78] = _pack_bands(inp["w5"], 4, [(0, 13)], 26, 52)
    b6m = np.zeros((26, 26), np.float32)
    for bb in range(2):
        for ch in range(13):
            for r in range(2):
                b6m[ch * 2 + r, bb * 13 + ch] = inp["w6"][ch, 0, r, bb]
    p16[0:26, O_B6:O_B6 + 26] = b6m
    # KAN layer 1 rhs pieces: silu part [13,20]; basis+bias part [105,20]
    p16[0:13, O_C1A:O_C1A + 20] = inp["sb1"]
    c1b = np.zeros((105, HIDDEN), np.float32)
    for n in range(8):
        for i in range(13):
            c1b[n * 13 + i] = inp["coef1"][i, :, n] * inp["ss1"][i] / 6.0
    c1b[104] = inp["bias1"]
    p16[0:105, O_C1B:O_C1B + 20] = c1b
    # KAN layer 2: silu [20,10]; basis rows 0..107 [108,10]; 108..159+bias [53,10]
    p16[0:20, O_C2S:O_C2S + 10] = inp["sb2"]
    c2b = np.zeros((161, NCLS), np.float32)
    for n in range(8):
        for i in range(20):
            c2b[n * 20 + i] = inp["coef2"][i, :, n] * inp["ss2"][i] / 6.0
    c2b[160] = inp["bias2"]
    p16[0:108, O_C2B1:O_C2B1 + 10] = c2b[0:108]
    p16[0:53, O_C2B2:O_C2B2 + 10] = c2b[108:161]
    # D = [u;1].T @ P_D gives D[img, k*13+ch] = u[ch,img] - k in one matmul
    pd = np.zeros((14, 156), np.float32)
    for k in range(12):
        for ch in range(13):
            pd[ch, k * 13 + ch] = 1.0
            pd[13, k * 13 + ch] = float(-k)
    p16[0:14, O_PD:O_PD + 156] = pd
    # D2 = [h1;1].T @ P_D2 with the u2 = 2.5*h1 + 5.5 affine folded in
    pd2 = np.zeros((21, 240), np.float32)
    for k in range(12):
        for ch in range(20):
            pd2[ch, k * 20 + ch] = 2.5
            pd2[20, k * 20 + ch] = 5.5 - float(k)
    p16[0:21, O_PD2:O_PD2 + 240] = pd2

    p32 = np.zeros((128, W32), np.float32)
    bv1 = _pack_bias(inp["b1"], CG1, 32, 64)
    for j in range(3):
        p32[0:64, O_BR1 + j] = bv1[:, 2 * j]
        p32[64:128, O_BR1 + j] = bv1[:, 2 * j + 1]
    p32[0:32, O_BR1 + 3] = bv1[0:32, 6]
    bv2 = _pack_bias(inp["b2"], CG2, 16, 64)
    p32[0:64, O_BR2] = bv2[:, 0]
    p32[64:128, O_BR2] = bv2[:, 1]
    p32[0:64, O_BR2 + 1] = bv2[:, 2]
    p32[64:80, O_BR2 + 1] = bv2[0:16, 3]
    bv3 = _pack_bias(inp["b3"], CG3, 8, 64)
    p32[0:64, O_BR3] = bv3[:, 0]
    p32[64:104, O_BR3] = bv3[0:40, 1]
    p32[0:52, O_BV4:O_BV4 + 1] = _pack_bias(inp["b4"], [(0, 13)], 4, 52)
    p32[0:26, O_BV5:O_BV5 + 1] = _pack_bias(inp["b5"], [(0, 13)], 2, 26)
    p32[0:13, O_UB:O_UB + 1] = (2.5 * (inp["b6"] + 2.2)).reshape(13, 1)
    p32[:, O_IDEN:O_IDEN + 128] = np.eye(128, dtype=np.float32)
    p32[:, O_MISC] = 2.2
    p32[:, O_MISC + 1] = 0.0
    p32[:, O_MISC + 2] = -2.2
    return {"c16": p16.astype(np.float16), "c32": p32}


def _shard_x(x_shard):
    # [256,13,64,64] -> xa [NG,128,6,GI,64] fp16 (partition = pair-slot*64+row),
    # xb [NG,64,GI,64] fp16 (ch 12)
    xs = x_shard.reshape(NG, GI, 13, 64, 64)
    a = xs[:, :, 0:12].reshape(NG, GI, 6, 2, 64, 64)
    a = a.transpose(0, 3, 4, 2, 1, 5).reshape(NG, 128, 6, GI, 64)
    xa = np.ascontiguousarray(a, dtype=np.float16)
    xb = np.ascontiguousarray(xs[:, :, 12].transpose(0, 2, 1, 3), dtype=np.float16)
    return xa, xb


# ----------------------------------------------------------------------------
# bass program
# ----------------------------------------------------------------------------

def _build():
    global _BUILT
    if _BUILT is not None:
        return _BUILT
    from contextlib import ExitStack
    import concourse.bass as bass  # noqa: F401
    import concourse.bacc as bacc
    import concourse.tile as tile
    import concourse.mybir as mybir

    f32 = mybir.dt.float32
    f16 = mybir.dt.float16
    AF = mybir.ActivationFunctionType
    OP = mybir.AluOpType
    AX = mybir.AxisListType

    nc = bacc.Bacc("TRN2")
    T = nc.tensor

    d_xa = nc.dram_tensor("xa", [NG, 128, 6, GI, 64], f16, kind="ExternalInput")
    d_xb = nc.dram_tensor("xb", [NG, 64, GI, 64], f16, kind="ExternalInput")
    d_c16 = nc.dram_tensor("c16", [128, W16], f16, kind="ExternalInput")
    d_c32 = nc.dram_tensor("c32", [128, W32], f32, kind="ExternalInput")
    d_out = nc.dram_tensor("out", [B, NCLS], f32, kind="ExternalOutput")

    PGI = 2 * GI  # images per pair

    with tile.TileContext(nc) as tc, ExitStack() as ctx:
        cpool = ctx.enter_context(tc.tile_pool(name="consts", bufs=1))
        c16 = cpool.tile([128, W16], f16, name="t_c16")
        c32 = cpool.tile([128, W32], f32, name="t_c32")
        # t_u rows 0-12: per-channel u values; row 13: ones (for the D matmul)
        t_u = cpool.tile([14, B], f16, name="t_u")

        p_x1 = ctx.enter_context(tc.tile_pool(name="x1", bufs=4))
        p_x2 = ctx.enter_context(tc.tile_pool(name="x2", bufs=2))
        p_x3 = ctx.enter_context(tc.tile_pool(name="x3", bufs=2))
        p_sm = ctx.enter_context(tc.tile_pool(name="xsm", bufs=2))
        kpool = ctx.enter_context(tc.tile_pool(name="kan", bufs=2))

        pp1 = ctx.enter_context(tc.tile_pool(name="ps1", bufs=3, space="PSUM"))
        pp2 = ctx.enter_context(tc.tile_pool(name="ps2", bufs=2, space="PSUM"))
        pp3 = ctx.enter_context(tc.tile_pool(name="ps3", bufs=1, space="PSUM"))
        pp456 = ctx.enter_context(tc.tile_pool(name="ps456", bufs=1, space="PSUM"))
        ppk = ctx.enter_context(tc.tile_pool(name="psk", bufs=1, space="PSUM"))

        def scrap_mm(src):
            # scrap matmul absorbing a DMA wait on the PE queue; shares the
            # conv3 PSUM bank (write-only, no readers -> no real conflicts)
            s = pp3.tile([104, PGI * 8], f32, tag="ps3", name="scrap")
            T.matmul(s[0:1, 0:1], src, src, start=True, stop=True)

        def bias_relu(e, dest, src, bap):
            if e == "s":
                nc.scalar.activation(dest, src, AF.Relu, bias=bap)
            else:
                nc.vector.tensor_scalar(dest, src, bap, 0.0,
                                        op0=OP.add, op1=OP.max)

        def group_dma(g):
            xt = p_x1.tile([128, 6 * GI * 64], f16, tag="x1", name="xt")
            h = 3 * GI * 64
            nc.sync.dma_start(
                xt[:, 0:h].rearrange("p (c i w) -> p c i w", c=3, w=64),
                d_xa[g, :, 0:3])
            nc.sync.dma_start(
                xt[:, h:2 * h].rearrange("p (c i w) -> p c i w", c=3, w=64),
                d_xa[g, :, 3:6])
            xtb = p_x1.tile([64, GI * 64], f16, tag="xb", name="xtb")
            nc.sync.dma_start(
                xtb[:, :].rearrange("p (i w) -> p i w", w=64), d_xb[g])
            return xt, xtb

        def conv1_group(xt, xtb, x2t, gq):
            # one group's conv1; channel-group pairs share a [128,512] PSUM
            # tile via the PE output partition offset
            csl = slice(gq * GI * 32, (gq + 1) * GI * 32)
            for j in range(4):
                cgs = [2 * j] if j == 3 else [2 * j, 2 * j + 1]
                ps = pp1.tile([128, GI * 32], f32, tag="ps1", name="ps1t")
                Mtot = 0
                for q, cg in enumerate(cgs):
                    nch = CG1[cg][1]
                    K, M = nch * 64, nch * 32
                    Mtot = 64 * q + M
                    if nch == 2:
                        xv = xt[0:K, cg * GI * 64:(cg + 1) * GI * 64].rearrange(
                            "p (i w) -> p i w", w=64)
                    else:
                        xv = xtb[0:K, :].rearrange("p (i w) -> p i w", w=64)
                    pv = ps[64 * q:64 * q + M, :].rearrange(
                        "p (i w) -> p i w", w=32)
                    lo = lambda b: O_B1 + (cg * 3 + b) * 64
                    T.matmul(pv, c16[0:K, lo(1):lo(1) + M],
                             xv[:, :, 0:64:2], start=True, stop=False)
                    T.matmul(pv, c16[0:K, lo(2):lo(2) + M],
                             xv[:, :, 1:64:2], start=False, stop=False)
                    T.matmul(pv[:, :, 1:32], c16[0:K, lo(0):lo(0) + M],
                             xv[:, :, 1:62:2], start=False, stop=True,
                             skip_group_check=True)
                bias_relu("s" if (j + gq) % 2 else "v",
                          x2t[j][0:Mtot, csl], ps[0:Mtot, :],
                          c32[0:Mtot, O_BR1 + j:O_BR1 + j + 1])

        # group-0/1 input DMAs + consts first; c16 gates the first matmul
        nc.sync.dma_start(c16[:, :], d_c16[:, :])
        g0_t = group_dma(0)
        nc.sync.dma_start(c32[:, :], d_c32[:, :])
        g01_tiles = [g0_t, group_dma(1)]
        scrap_mm(c16[0:1, 0:1])
        scrap_mm(c32[0:1, 0:1])
        # ones row for the KAN D matmul; conv6 overwrites rows 0-12
        nc.vector.memset(t_u[0:14, :], 1.0)

        # ------------------------------ KAN head ------------------------------
        def emit_kan(t):
            """Generator: one KAN tile (128 images) in stages; yields between
            stages so tile 0 can interleave with conv pairs 4-7."""
            sl = slice(t * 128, (t + 1) * 128)
            ps_D = ppk.tile([128, 256], f32, tag="kps", name="ps_D")
            T.matmul(ps_D[0:128, 0:156], t_u[0:14, sl],
                     c16[0:14, O_PD:O_PD + 156], start=True, stop=True)
            D = kpool.tile([128, 156], f16, tag="D", name="Dt")
            nc.vector.tensor_copy(D[:, :], ps_D[0:128, 0:156])
            ge = kpool.tile([128, 143], f16, tag="ge", name="ge")
            lt = kpool.tile([128, 143], f16, tag="lt", name="lt")
            nc.vector.tensor_scalar(ge[:, :], D[:, 0:143], 0.0, None, op0=OP.is_ge)
            nc.vector.tensor_scalar(lt[:, :], D[:, 13:156], 0.0, None, op0=OP.is_lt)
            Bc = kpool.tile([128, 143], f16, tag="B0", name="Bc")
            nc.vector.tensor_mul(Bc[:, :], ge[:, :], lt[:, :])
            yield
            # Cox-de-Boor levels (unnormalized; /6 folded into c1b); the last
            # level is written fp32 because it feeds an fp32 PE transpose
            wid = 143
            for p in range(1, 4):
                wid -= 13
                dt_l = f16 if p < 3 else f32
                ta = kpool.tile([128, wid], dt_l, tag=f"ta{p}", name="ta")
                tb = kpool.tile([128, wid], dt_l, tag=f"tb{p}", name="tb")
                nc.vector.tensor_mul(ta[:, :], D[:, 0:wid], Bc[:, 0:wid])
                nc.vector.tensor_mul(tb[:, :], D[:, 13 * (p + 1):13 * (p + 1) + wid],
                                     Bc[:, 13:13 + wid])
                if p < 3:
                    Bc = kpool.tile([128, wid], f16, tag=f"B{p}", name="Bc")
                    nc.vector.tensor_sub(Bc[:, :], ta[:, :], tb[:, :])
                else:
                    Bc = kpool.tile([128, 105], f32, tag="B3", name="Bc")
                    nc.vector.tensor_sub(Bc[:, 0:104], ta[:, :], tb[:, :])
                    nc.vector.memset(Bc[:, 104:105], 1.0)
            # stkA = silu(x) = x / (1 + exp(-x)), x = 0.4*u - 2.2; keeps every
            # ACT function in the natural_log_exp table set (no reloads)
            xA = kpool.tile([13, 128], f32, tag="xA", name="xA")
            nc.vector.tensor_scalar(xA[:, :], t_u[0:13, sl], H_GRID, -2.2,
                                    op0=OP.mult, op1=OP.add)
            eA = kpool.tile([13, 128], f32, tag="eA", name="eA")
            nc.scalar.activation(eA[:, :], t_u[0:13, sl], AF.Exp,
                                 bias=c32[0:13, O_MISC:O_MISC + 1],
                                 scale=-H_GRID)
            dA = kpool.tile([13, 128], f32, tag="dA", name="dA")
            nc.vector.tensor_scalar(dA[:, :], eA[:, :], 1.0, None, op0=OP.add)
            rA = kpool.tile([13, 128], f32, tag="rA", name="rA")
            nc.vector.reciprocal(rA[:, :], dA[:, :])
            stkA = kpool.tile([13, 128], f16, tag="stkA", name="stkA")
            nc.vector.tensor_mul(stkA[:, :], xA[:, :], rA[:, :])
            yield
            stkB = kpool.tile([105, 128], f16, tag="stkB", name="stkB")
            ps_b1 = ppk.tile([128, 256], f32, tag="kps", name="ps_b1")
            T.transpose(ps_b1[0:105, 0:128], Bc[:, 0:105],
                        c32[:, O_IDEN:O_IDEN + 128])
            nc.scalar.copy(stkB[:, :], ps_b1[0:105, 0:128])
            yield
            ps_h1 = ppk.tile([128, 256], f32, tag="kps", name="ps_h1")
            T.matmul(ps_h1[0:128, 0:20], stkA[:, :], c16[0:13, O_C1A:O_C1A + 20],
                     start=True, stop=False)
            T.matmul(ps_h1[0:128, 0:20], stkB[:, :], c16[0:105, O_C1B:O_C1B + 20],
                     start=False, stop=True)
            yield
            # ---- KAN layer 2 (u2 = 2.5*h1 + 5.5 folded into P_D2) ----
            h1 = kpool.tile([128, 20], f32, tag="h1", name="h1")
            nc.scalar.copy(h1[:, :], ps_h1[0:128, 0:20])
            ps_t2 = ppk.tile([128, 256], f32, tag="kps", name="ps_t2")
            T.transpose(ps_t2[0:20, 0:128], h1[:, :], c32[:, O_IDEN:O_IDEN + 128])
            stk2u = kpool.tile([21, 128], f16, tag="s2u", name="stk2u")
            nc.vector.memset(stk2u[0:21, :], 1.0)
            nc.scalar.copy(stk2u[0:20, :], ps_t2[0:20, 0:128])
            # s2s = silu(h1) (the reference's base term for layer 2)
            e2 = kpool.tile([20, 128], f32, tag="e2", name="e2")
            nc.scalar.activation(e2[:, :], stk2u[0:20, :], AF.Exp,
                                 bias=c32[0:20, O_MISC + 1:O_MISC + 2],
                                 scale=-1.0)
            d2 = kpool.tile([20, 128], f32, tag="d2", name="d2")
            nc.vector.tensor_scalar(d2[:, :], e2[:, :], 1.0, None, op0=OP.add)
            r2 = kpool.tile([20, 128], f32, tag="r2", name="r2")
            nc.vector.reciprocal(r2[:, :], d2[:, :])
            stk2s = kpool.tile([20, 128], f16, tag="s2s", name="stk2s")
            nc.vector.tensor_mul(stk2s[:, :], stk2u[0:20, :], r2[:, :])
            ps_D2 = ppk.tile([128, 256], f32, tag="kps", name="ps_D2")
            T.matmul(ps_D2[0:128, 0:240], stk2u[0:21, :],
                     c16[0:21, O_PD2:O_PD2 + 240], start=True, stop=True)
            D2 = kpool.tile([128, 240], f16, tag="D2", name="D2t")
            nc.vector.tensor_copy(D2[:, :], ps_D2[0:128, 0:240])
            ge2 = kpool.tile([128, 220], f16, tag="ge2", name="ge2")
            lt2 = kpool.tile([128, 220], f16, tag="lt2", name="lt2")
            nc.vector.tensor_scalar(ge2[:, :], D2[:, 0:220], 0.0, None, op0=OP.is_ge)
            nc.vector.tensor_scalar(lt2[:, :], D2[:, 20:240], 0.0, None, op0=OP.is_lt)
            Bc2 = kpool.tile([128, 220], f16, tag="B0_2", name="Bc2")
            nc.vector.tensor_mul(Bc2[:, :], ge2[:, :], lt2[:, :])
            yield
            wid = 220
            for p in range(1, 4):
                wid -= 20
                dt_l = f16 if p < 3 else f32
                ta = kpool.tile([128, wid], dt_l, tag=f"t2a{p}", name="ta2")
                tb = kpool.tile([128, wid], dt_l, tag=f"t2b{p}", name="tb2")
                nc.vector.tensor_mul(ta[:, :], D2[:, 0:wid], Bc2[:, 0:wid])
                nc.vector.tensor_mul(tb[:, :], D2[:, 20 * (p + 1):20 * (p + 1) + wid],
                                     Bc2[:, 20:20 + wid])
                if p < 3:
                    Bc2 = kpool.tile([128, wid], f16, tag=f"B{p}_2", name="Bc2")
                    nc.vector.tensor_sub(Bc2[:, :], ta[:, :], tb[:, :])
                else:
                    Bc2 = kpool.tile([128, 161], f32, tag="B3_2", name="Bc2")
                    nc.vector.tensor_sub(Bc2[:, 0:160], ta[:, :], tb[:, :])
                    nc.vector.memset(Bc2[:, 160:161], 1.0)
            yield
            stk2a = kpool.tile([108, 128], f16, tag="s2a", name="stk2a")
            stk2b = kpool.tile([53, 128], f16, tag="s2b", name="stk2b")
            ps_b2 = ppk.tile([128, 256], f32, tag="kps", name="ps_b2")
            T.transpose(ps_b2[0:108, 0:128], Bc2[:, 0:108],
                        c32[:, O_IDEN:O_IDEN + 128])
            nc.scalar.copy(stk2a[:, :], ps_b2[0:108, 0:128])
            ps_b3 = ppk.tile([128, 256], f32, tag="kps", name="ps_b3")
            T.transpose(ps_b3[0:53, 0:128], Bc2[:, 108:161],
                        c32[:, O_IDEN:O_IDEN + 128])
            nc.vector.tensor_copy(stk2b[:, :], ps_b3[0:53, 0:128])
            yield
            ps_lg = ppk.tile([128, 256], f32, tag="kps", name="ps_lg")
            T.matmul(ps_lg[0:128, 0:NCLS], stk2a[:, :],
                     c16[0:108, O_C2B1:O_C2B1 + 10], start=True, stop=False)
            T.matmul(ps_lg[0:128, 0:NCLS], stk2s[:, :],
                     c16[0:20, O_C2S:O_C2S + 10], start=False, stop=False)
            T.matmul(ps_lg[0:128, 0:NCLS], stk2b[:, :],
                     c16[0:53, O_C2B2:O_C2B2 + 10], start=False, stop=True)
            yield
            # ---- log_softmax (on an SBUF copy) ----
            lg_s = kpool.tile([128, NCLS], f32, tag="lg_s", name="lg_s")
            nc.vector.tensor_copy(lg_s[:, :], ps_lg[0:128, 0:NCLS])
            negm = kpool.tile([128, 1], f32, tag="negm", name="negm")
            nc.vector.reduce_max(negm[:, :], lg_s[:, :], axis=AX.X, negate=True)
            ex = kpool.tile([128, NCLS], f32, tag="ex", name="ex")
            nc.scalar.activation(ex[:, :], lg_s[:, :], AF.Exp, bias=negm[:, 0:1])
            ssum = kpool.tile([128, 1], f32, tag="ssum", name="ssum")
            nc.vector.reduce_sum(ssum[:, :], ex[:, :], axis=AX.X)
            lsum = kpool.tile([128, 1], f32, tag="lsum", name="lsum")
            nc.scalar.activation(lsum[:, :], ssum[:, :], AF.Ln,
                                 bias=c32[0:128, O_MISC + 1:O_MISC + 2])
            res = kpool.tile([128, NCLS], f32, tag="res", name="res")
            nc.vector.tensor_scalar(res[:, :], lg_s[:, :], negm[:, 0:1],
                                    lsum[:, 0:1], op0=OP.add, op1=OP.subtract)
            nc.sync.dma_start(d_out[sl, :], res[:, :])

        kan_gens = []
        for gp in range(NP):
            g0, g1 = 2 * gp, 2 * gp + 1
            pr = [g01_tiles[0], g01_tiles[1]] if gp == 0 else \
                 [group_dma(g0), group_dma(g1)]
            psl = slice(g0 * GI, (g1 + 1) * GI)  # pair image slice

            # ---- conv1 (per group) -> x2 pair tiles ----
            x2t = [p_x2.tile([128, PGI * 32], f16, tag=f"x2_{k}", name=f"x2_{k}")
                   for k in range(4)]
            for gq in range(2):
                xt, xtb = pr[gq]
                h = 3 * GI * 64
                scrap_mm(xt[0:1, 0:1])
                scrap_mm(xt[0:1, h:h + 1])
                scrap_mm(xtb[0:1, 0:1])
                conv1_group(xt, xtb, x2t, gq)

            # ---- conv2 on the pair -> x3 tiles ----
            x3t = [p_x3.tile([128, PGI * 16], f16, tag=f"x3_{k}", name=f"x3_{k}")
                   for k in range(2)]
            for j in range(2):
                ps = pp2.tile([128, PGI * 16], f32, tag="ps2", name="ps2t")
                Mtot = 0
                for q, k4 in enumerate((2 * j, 2 * j + 1)):
                    nch = CG2[k4][1]
                    K, M = nch * 32, nch * 16
                    Mtot = 64 * q + M
                    xv = x2t[k4][0:K, :].rearrange("p (i w) -> p i w", w=32)
                    pv = ps[64 * q:64 * q + M, :].rearrange(
                        "p (i w) -> p i w", w=16)
                    lo = lambda b: O_B2 + (k4 * 3 + b) * 64
                    T.matmul(pv, c16[0:K, lo(1):lo(1) + M],
                             xv[:, :, 0:32:2], start=True, stop=False)
                    T.matmul(pv, c16[0:K, lo(2):lo(2) + M],
                             xv[:, :, 1:32:2], start=False, stop=False)
                    T.matmul(pv[:, :, 1:16], c16[0:K, lo(0):lo(0) + M],
                             xv[:, :, 1:30:2], start=False, stop=True,
                             skip_group_check=True)
                bias_relu("s" if (j + gp) % 2 else "v",
                          x3t[j][0:Mtot, :], ps[0:Mtot, :],
                          c32[0:Mtot, O_BR2 + j:O_BR2 + j + 1])

            # ---- conv3 on the pair -> x4 [104, PGI*8] ----
            x4 = p_sm.tile([104, PGI * 8], f16, tag="x4", name="x4")
            ps3 = pp3.tile([104, PGI * 8], f32, tag="ps3", name="ps3t")
            for q, k8 in enumerate((0, 1)):
                nch = CG3[k8][1]
                K, M = nch * 16, nch * 8
                xv = x3t[k8][0:K, :].rearrange("p (i w) -> p i w", w=16)
                pv = ps3[64 * q:64 * q + M, :].rearrange("p (i w) -> p i w", w=8)
                lo = lambda b: O_B3 + (k8 * 3 + b) * 64
                T.matmul(pv, c16[0:K, lo(1):lo(1) + M],
                         xv[:, :, 0:16:2], start=True, stop=False)
                T.matmul(pv, c16[0:K, lo(2):lo(2) + M],
                         xv[:, :, 1:16:2], start=False, stop=False)
                T.matmul(pv[:, :, 1:8], c16[0:K, lo(0):lo(0) + M],
                         xv[:, :, 1:14:2], start=False, stop=True,
                         skip_group_check=True)
            bias_relu("s" if gp % 2 else "v", x4[0:104, :], ps3[0:104, :],
                      c32[0:104, O_BR3:O_BR3 + 1])

            # ---- conv4 [104 -> 52] ----
            x5 = p_sm.tile([52, PGI * 4], f16, tag="x5", name="x5")
            xv = x4[0:104, :].rearrange("p (i w) -> p i w", w=8)
            ps4 = pp456.tile([64, PGI * 4], f32, tag="ps456", name="ps4t")
            pv = ps4[0:52, :].rearrange("p (i w) -> p i w", w=4)
            T.matmul(pv, c16[0:104, O_B4 + 52:O_B4 + 104], xv[:, :, 0:8:2],
                     start=True, stop=False)
            T.matmul(pv, c16[0:104, O_B4 + 104:O_B4 + 156], xv[:, :, 1:8:2],
                     start=False, stop=False)
            T.matmul(pv[:, :, 1:4], c16[0:104, O_B4:O_B4 + 52],
                     xv[:, :, 1:6:2], start=False, stop=True,
                     skip_group_check=True)
            bias_relu("v", x5[:, :], ps4[0:52, :], c32[0:52, O_BV4:O_BV4 + 1])

            # ---- conv5 [52 -> 26] ----
            x6 = p_sm.tile([26, PGI * 2], f16, tag="x6", name="x6")
            xv = x5[0:52, :].rearrange("p (i w) -> p i w", w=4)
            ps5 = pp456.tile([64, PGI * 2], f32, tag="ps456", name="ps5t")
            pv = ps5[0:26, :].rearrange("p (i w) -> p i w", w=2)
            T.matmul(pv, c16[0:52, O_B5 + 26:O_B5 + 52], xv[:, :, 0:4:2],
                     start=True, stop=False)
            T.matmul(pv, c16[0:52, O_B5 + 52:O_B5 + 78], xv[:, :, 1:4:2],
                     start=False, stop=False)
            T.matmul(pv[:, :, 1:2], c16[0:52, O_B5:O_B5 + 26],
                     xv[:, :, 1:2:2], start=False, stop=True,
                     skip_group_check=True)
            bias_relu("s", x6[:, :], ps5[0:26, :], c32[0:26, O_BV5:O_BV5 + 1])

            # ---- conv6 (2x2 valid) -> u[:, pair slice] ----
            xv = x6[0:26, :].rearrange("p (i w) -> p i w", w=2)
            ps6 = pp456.tile([64, PGI], f32, tag="ps456", name="ps6t")
            T.matmul(ps6[0:13, :], c16[0:26, O_B6:O_B6 + 13],
                     xv[:, :, 0:1], start=True, stop=False)
            T.matmul(ps6[0:13, :], c16[0:26, O_B6 + 13:O_B6 + 26],
                     xv[:, :, 1:2], start=False, stop=True)
            nc.vector.tensor_scalar(t_u[0:13, psl], ps6[0:13, :],
                                    2.5, c32[0:13, O_UB:O_UB + 1],
                                    op0=OP.mult, op1=OP.add)

            if gp == 3:
                kan_gens.append(emit_kan(0))
            for gen in kan_gens:
                next(gen, None)
                next(gen, None)

        kan_gens.append(emit_kan(1))
        for gen in kan_gens:
            for _ in gen:
                pass

    nc.compile()  # bacc lowering: wait splitting via event semaphores, etc.
    _BUILT = (nc, ["xa", "xb", "c16", "c32"])
    return _BUILT


# ----------------------------------------------------------------------------
# entry point
# ----------------------------------------------------------------------------

def kernel(**inputs):
    from concourse import bass_utils

    x = np.asarray(inputs["x"], np.float32)
    cons = _host_consts({k: np.asarray(v, np.float32)
                         for k, v in inputs.items() if k != "x"})
    nc, _names = _build()

    in_maps = []
    for core in range(NCORE):
        xa, xb = _shard_x(x[core * B:(core + 1) * B])
        in_maps.append({"xa": xa, "xb": xb, **cons})
    res = bass_utils.run_bass_kernel_spmd(nc, in_maps, core_ids=list(range(NCORE)))
    global LAST_RES
    LAST_RES = res
    return np.concatenate([r["out"] for r in res.results], axis=0)


LAST_RES = None
